# revision 7
# baseline (speedup 1.0000x reference)
"""Trainium2 Bass kernel for CoreRNNFW (fast-weight RNN).

Key ideas:
- Pure data parallel: B=32 batches sharded 4-per-core across 8 cores.
- The fast-weight matrix A is never materialized. Since A_t =
  eta * sum_{s<t} lambda^(t-1-s) h_s h_s^T, the inner-read matvec A@h is
  computed from the history of committed h vectors:
      c[s]  = <h_s, h>              (PE matmul against history-transpose)
      c'[s] = eta*lambda^(t-1-s)*c[s] (one DVE mult with a host-built table)
      A@h   = sum_s c'[s] h_s        (PE matmul against history-rows)
  This replaces O(d_h^2) per-batch work with O(T*d_h).
- d_h is stored interleaved: index j <-> (p, f) with j = p*4 + f so that a
  [128, 4]-per-batch tile is exactly the GPSIMD fused-layernorm striping
  (token = all 128 partitions, F=4), letting one gpsimd instruction do the
  whole LN (mean/var/rsqrt/gamma/beta) per batch.
- All fp32 throughout (PE streams one column per cycle regardless of dtype).
"""

import sys

sys.path.insert(0, "/opt/trn_rl_repo")

import numpy as np

import concourse.bacc as bacc
import concourse.mybir as mybir
from concourse import tile
from concourse import library_config
from concourse.bass_utils import run_bass_kernel_spmd

N_CORES = 8
T = 32          # sequence length
B = 32          # global batch
BL = 4          # batch per core
DG = 256        # input dim
DH = 512        # hidden dim
P = 128         # partitions
F = DH // P     # 4: free elems per partition for one hidden vector
S = 32          # history slots (steps 0..30 used, slot 31 spare)
LAMBDA = 0.95
ETA = 0.5
EPS = 1e-5
S_LOOP = 2

FP32 = mybir.dt.float32

_cached = None  # (nc, names)


def _build():
    nc = bacc.Bacc("TRN2", target_bir_lowering=False, debug=False)

    # DRAM I/O ----------------------------------------------------------
    # wh:  [pk, (f_k, f', p)] lhsT tiles of W_h^T (j-major K tiles)
    # wg:  [gg, (gc, f', p)] lhsT tiles of W_g^T
    # zt:  [gg, (gc, t, b)]  z transposed, rhs for the zW precompute
    # bh/gamma/beta: [p, f]
    # maskw: [(b,s), (t, b')] = delta_{b,b'} * eta * lambda^(t-1-s) (s<t)
    wh_d = nc.dram_tensor("wh", [P, 4, F, P], FP32, kind="ExternalInput")
    wg_d = nc.dram_tensor("wg", [P, 2, F, P], FP32, kind="ExternalInput")
    zt_d = nc.dram_tensor("zt", [P, 2, T, BL], FP32, kind="ExternalInput")
    bh_d = nc.dram_tensor("bh", [P, F], FP32, kind="ExternalInput")
    gam_d = nc.dram_tensor("gam", [P, F], FP32, kind="ExternalInput")
    bet_d = nc.dram_tensor("bet", [P, F], FP32, kind="ExternalInput")
    msk_d = nc.dram_tensor("msk", [P, T, BL], FP32, kind="ExternalInput")
    out_d = nc.dram_tensor("out", [P, BL, F], FP32, kind="ExternalOutput")

    with tile.TileContext(nc) as tc:
        with (
            tc.tile_pool(name="state", bufs=1) as state,
            tc.tile_pool(name="xpool", bufs=3) as xpool,
            tc.tile_pool(name="cpool", bufs=3) as cpool,
            tc.tile_pool(name="pxp", bufs=2, space="PSUM") as pxp,
            tc.tile_pool(name="pcp", bufs=2, space="PSUM") as pcp,
        ):
            wh = state.tile([P, 4, F, P], FP32)     # 8KB/part
            wg = state.tile([P, 2, F, P], FP32)     # 4KB/part
            zt = state.tile([P, 2, T, BL], FP32)
            bh = state.tile([P, F], FP32)
            gam = state.tile([P, F], FP32)
            bet = state.tile([P, F], FP32)
            msk = state.tile([P, T, BL], FP32)
            zw = state.tile([P, T, BL, F], FP32)    # 2KB/part: W_g z + b_h
            HT = state.tile([P, F, BL, S], FP32)    # history^T: [p,(f,b,s)]
            Hs = state.tile([P, P, F], FP32)        # history rows: [(b,s),(p,f')]
            hcur = state.tile([P, BL, F], FP32)     # current h, [p,(b,f)]
            lno = state.tile([P, BL, F], FP32)      # layernorm output

            nc.gpsimd.load_library(library_config.attn)

            nc.sync.dma_start(wh[:], wh_d[:])
            nc.sync.dma_start(wg[:], wg_d[:])
            nc.sync.dma_start(zt[:], zt_d[:])
            nc.sync.dma_start(bh[:], bh_d[:])
            nc.sync.dma_start(gam[:], gam_d[:])
            nc.sync.dma_start(bet[:], bet_d[:])
            nc.sync.dma_start(msk[:], msk_d[:])

            nc.vector.memset(HT[:], 0.0)
            nc.gpsimd.memset(Hs[:], 0.0)

            # Precompute zw[t, b, :] = W_g z_t[b] + b_h  (as transposed layout)
            for fp in range(F):
                zwp = pxp.tile([P, T, BL], FP32, tag="zwp")
                for gc in range(2):
                    nc.tensor.matmul(
                        zwp[:],
                        wg[:, gc, fp, :],
                        zt[:, gc, :, :],
                        start=(gc == 0),
                        stop=(gc == 1),
                    )
                nc.vector.tensor_scalar_add(zw[:, :, :, fp], zwp[:], bh[:, fp : fp + 1])

            def layer_norm_relu(x_sb, t, last):
                """x_sb [P, BL, F] -> hcur (and HT slot t unless last)."""
                for b in range(BL):
                    nc.gpsimd.layernorm(
                        lno[:, b, :],
                        x_sb[:, b, :],
                        gamma_ap=gam[:],
                        beta_ap=bet[:],
                        eps=EPS,
                        subtract_mean=True,
                        n_tokens=1,
                    )
                nc.vector.tensor_relu(hcur[:], lno[:])
                if not last:
                    # committed h also goes into history^T (relu fused in ACT)
                    nc.scalar.activation(
                        HT[:, :, :, t],
                        lno.rearrange("p b f -> p f b"),
                        mybir.ActivationFunctionType.Relu,
                    )

            def append_rows(t):
                # Hs[(b, t), (p, f')] = hcur[p, b, f']  (one small DMA per b)
                for b in range(BL):
                    r = b * S + t
                    nc.sync.dma_start(Hs[r : r + 1], hcur[:, b, :])

            for t in range(T):
                last = t == T - 1
                if t == 0:
                    x0 = xpool.tile([P, BL, F], FP32, tag="x")
                    nc.vector.tensor_copy(x0[:], zw[:, 0, :, :])
                    layer_norm_relu(x0, t, last)
                    append_rows(t)
                    continue

                # h_base^T = W_h h_{t-1} (+ zw added on psum->sbuf copy)
                px = pxp.tile([P, F, BL], FP32, tag="px")
                for fp in range(F):
                    for fk in range(F):
                        # start arms zero-on-first-touch for the whole 2KB
                        # psum region: exactly one start per px lifetime.
                        nc.tensor.matmul(
                            px[:, fp, :],
                            wh[:, fk, fp, :],
                            hcur[:, :, fk],
                            start=(fp == 0 and fk == 0),
                            stop=(fp == F - 1 and fk == F - 1),
                        )
                x = xpool.tile([P, BL, F], FP32, tag="x")
                nc.vector.tensor_add(x[:], px.rearrange("p f b -> p b f"), zw[:, t, :, :])
                layer_norm_relu(x, t, True)  # k=0: do not commit yet

                c_prev = None
                for k in range(S_LOOP):
                    lastk = k == S_LOOP - 1
                    # c^T[(b,s), b'] = sum_j H^T[j,(b,s)] h[j, b']
                    pct = pcp.tile([P, BL], FP32, tag="pct")
                    for f in range(F):
                        nc.tensor.matmul(
                            pct[:],
                            HT[:, f, :, :],
                            hcur[:, :, f],
                            start=(f == 0),
                            stop=(f == F - 1),
                        )
                    ck = cpool.tile([P, BL], FP32, tag="ck")
                    nc.vector.tensor_mul(ck[:], pct[:], msk[:, t, :])
                    if c_prev is None:
                        rhs = ck
                    else:
                        dp = cpool.tile([P, BL], FP32, tag="dp")
                        nc.vector.tensor_sub(dp[:], ck[:], c_prev[:])
                        rhs = dp
                    c_prev = ck
                    # x^T += Ah^T (delta form so px keeps h_base + latest Ah)
                    for fp in range(F):
                        nc.tensor.matmul(
                            px[:, fp, :],
                            Hs[:, :, fp],
                            rhs[:],
                            start=False,
                            stop=(fp == F - 1),
                            skip_group_check=True,
                        )
                    xk = xpool.tile([P, BL, F], FP32, tag="x")
                    nc.vector.tensor_add(xk[:], px.rearrange("p f b -> p b f"), zw[:, t, :, :])
                    layer_norm_relu(xk, t, last or not lastk)

                if not last:
                    append_rows(t)

            nc.sync.dma_start(out_d[:], hcur[:])

    nc.compile()
    return nc


def _host_prep(z_seq, W_h, W_g, b_h, ln_gamma, ln_beta):
    """Build the per-core input maps (all layout shuffling happens here)."""
    z_seq = np.asarray(z_seq, np.float32)
    W_h = np.ascontiguousarray(np.asarray(W_h, np.float32))
    W_g = np.ascontiguousarray(np.asarray(W_g, np.float32))
    b_h = np.asarray(b_h, np.float32)
    ln_gamma = np.asarray(ln_gamma, np.float32)
    ln_beta = np.asarray(ln_beta, np.float32)

    # lhsT tiles: wh[pk, f_k, f', p] = W_h[p*4+f', pk*4+f_k]
    wh = np.ascontiguousarray(
        W_h.reshape(P, F, P, F).transpose(2, 3, 1, 0)
    )
    # wg[gg, gc, f', p] = W_g[p*4+f', gc*128+gg]
    wg = np.ascontiguousarray(
        W_g.reshape(P, F, 2, P).transpose(3, 2, 1, 0)
    )
    bh = np.ascontiguousarray(b_h.reshape(P, F))
    gam = np.ascontiguousarray(ln_gamma.reshape(P, F))
    bet = np.ascontiguousarray(ln_beta.reshape(P, F))

    # maskw[(b,s), (t, b')] = (b==b') * eta * lambda^(t-1-s) for s<t else 0
    msk = np.zeros((BL, S, T, BL), np.float64)
    for t in range(1, T):
        s = np.arange(t)
        w = ETA * LAMBDA ** (t - 1 - s)
        for b in range(BL):
            msk[b, :t, t, b] = w
    msk = np.ascontiguousarray(msk.reshape(P, T, BL).astype(np.float32))

    in_maps = []
    for c in range(N_CORES):
        zl = z_seq[:, c * BL : (c + 1) * BL, :]  # [T, BL, DG]
        # zt[gg, gc, t, b] = z[t, b, gc*128+gg]
        zt = np.ascontiguousarray(
            zl.transpose(2, 0, 1).reshape(2, P, T, BL).transpose(1, 0, 2, 3)
        )
        in_maps.append(
            {
                "wh": wh, "wg": wg, "zt": zt, "bh": bh,
                "gam": gam, "bet": bet, "msk": msk,
            }
        )
    return in_maps


def kernel(z_seq, W_h, W_g, b_h, ln_gamma, ln_beta):
    global _cached
    if _cached is None:
        _cached = _build()
    nc = _cached
    in_maps = _host_prep(z_seq, W_h, W_g, b_h, ln_gamma, ln_beta)
    res = run_bass_kernel_spmd(nc, in_maps, core_ids=list(range(N_CORES)))
    outs = []
    for c in range(N_CORES):
        raw = res.results[c]["out"]  # [P, BL, F]: raw[p, b, f] = h[b, p*4+f]
        outs.append(raw.transpose(1, 0, 2).reshape(BL, DH))
    return np.ascontiguousarray(np.concatenate(outs, axis=0).astype(np.float32))


# revision 8
# speedup vs baseline: 1.7764x; 1.7764x over previous
"""Trainium2 Bass kernel for CoreRNNFW (fast-weight RNN).

Key ideas:
- Pure data parallel: B=32 batches sharded 4-per-core across 8 cores.
- The fast-weight matrix A is never materialized. Since A_t =
  eta * sum_{s<t} lambda^(t-1-s) h_s h_s^T, the inner-read matvec A@h is
  computed from the history of committed h vectors:
      c[s]  = <h_s, h>              (PE matmul against history-transpose)
      c'[s] = eta*lambda^(t-1-s)*c[s] (one DVE mult with a host-built table)
      A@h   = sum_s c'[s] h_s        (PE matmul against history-rows)
  This replaces O(d_h^2) per-batch work with O(T*d_h).
- d_h is stored interleaved: index j <-> (p, f) with j = p*4 + f so that a
  [128, 4]-per-batch tile is exactly the GPSIMD fused-layernorm striping
  (token = all 128 partitions, F=4), letting one gpsimd instruction do the
  whole LN (mean/var/rsqrt/gamma/beta) per batch.
- All fp32 throughout (PE streams one column per cycle regardless of dtype).
"""

import sys

sys.path.insert(0, "/opt/trn_rl_repo")

import numpy as np

import concourse.bacc as bacc
import concourse.mybir as mybir
from concourse import tile
from concourse import library_config
from concourse.bass_utils import run_bass_kernel_spmd

N_CORES = 8
T = 32          # sequence length
B = 32          # global batch
BL = 4          # batch per core
DG = 256        # input dim
DH = 512        # hidden dim
P = 128         # partitions
F = DH // P     # 4: free elems per partition for one hidden vector
S = 32          # history slots (steps 0..30 used, slot 31 spare)
LAMBDA = 0.95
ETA = 0.5
EPS = 1e-5
S_LOOP = 2

FP32 = mybir.dt.float32

_cached = None  # (nc, names)


def _build():
    nc = bacc.Bacc("TRN2", target_bir_lowering=False, debug=False)

    # DRAM I/O ----------------------------------------------------------
    # wh:  [pk, (f_k, f', p)] lhsT tiles of W_h^T (j-major K tiles)
    # wg:  [gg, (gc, f', p)] lhsT tiles of W_g^T
    # zt:  [gg, (gc, t, b)]  z transposed, rhs for the zW precompute
    # bh/gamma/beta: [p, f]
    # maskw: [(b,s), (t, b')] = delta_{b,b'} * eta * lambda^(t-1-s) (s<t)
    wh_d = nc.dram_tensor("wh", [P, 4, F, P], FP32, kind="ExternalInput")
    wg_d = nc.dram_tensor("wg", [P, 2, F, P], FP32, kind="ExternalInput")
    zt_d = nc.dram_tensor("zt", [P, 2, T, BL], FP32, kind="ExternalInput")
    bh_d = nc.dram_tensor("bh", [P, F], FP32, kind="ExternalInput")
    gam_d = nc.dram_tensor("gam", [P, F], FP32, kind="ExternalInput")
    bet_d = nc.dram_tensor("bet", [P, F], FP32, kind="ExternalInput")
    msk_d = nc.dram_tensor("msk", [P, T, BL], FP32, kind="ExternalInput")
    out_d = nc.dram_tensor("out", [P, BL, F], FP32, kind="ExternalOutput")

    with tile.TileContext(nc) as tc:
        with (
            tc.tile_pool(name="state", bufs=1) as state,
            tc.tile_pool(name="xpool", bufs=3) as xpool,
            tc.tile_pool(name="cpool", bufs=3) as cpool,
            tc.tile_pool(name="pxp", bufs=2, space="PSUM") as pxp,
            tc.tile_pool(name="pcp", bufs=2, space="PSUM") as pcp,
        ):
            wh = state.tile([P, 4, F, P], FP32)     # 8KB/part
            wg = state.tile([P, 2, F, P], FP32)     # 4KB/part
            zt = state.tile([P, 2, T, BL], FP32)
            bh = state.tile([P, F], FP32)
            gam = state.tile([P, F], FP32)
            bet = state.tile([P, F], FP32)
            msk = state.tile([P, T, BL], FP32)
            zw = state.tile([P, T, BL, F], FP32)    # 2KB/part: W_g z + b_h
            HT = state.tile([P, F, BL, S], FP32)    # history^T: [p,(f,b,s)]
            Hs = state.tile([P, P, F], FP32)        # history rows: [(b,s),(p,f')]
            hcur = state.tile([P, BL, F], FP32)     # current h, [p,(b,f)]
            lno = state.tile([P, BL, F], FP32)      # layernorm output

            nc.gpsimd.load_library(library_config.attn)

            nc.sync.dma_start(wh[:], wh_d[:])
            nc.sync.dma_start(wg[:], wg_d[:])
            nc.sync.dma_start(zt[:], zt_d[:])
            nc.sync.dma_start(bh[:], bh_d[:])
            nc.sync.dma_start(gam[:], gam_d[:])
            nc.sync.dma_start(bet[:], bet_d[:])
            nc.sync.dma_start(msk[:], msk_d[:])

            nc.vector.memset(HT[:], 0.0)
            nc.gpsimd.memset(Hs[:], 0.0)

            # Precompute zw[t, b, :] = W_g z_t[b] + b_h  (as transposed layout)
            for fp in range(F):
                zwp = pxp.tile([P, T, BL], FP32, tag="zwp")
                for gc in range(2):
                    nc.tensor.matmul(
                        zwp[:],
                        wg[:, gc, fp, :],
                        zt[:, gc, :, :],
                        start=(gc == 0),
                        stop=(gc == 1),
                    )
                nc.vector.tensor_scalar_add(zw[:, :, :, fp], zwp[:], bh[:, fp : fp + 1])

            def layer_norm_relu(x_sb, t, last):
                """x_sb [P, BL, F] -> hcur (and HT slot t unless last)."""
                for b in range(BL):
                    nc.gpsimd.layernorm(
                        lno[:, b, :],
                        x_sb[:, b, :],
                        gamma_ap=gam[:],
                        beta_ap=bet[:],
                        eps=EPS,
                        subtract_mean=True,
                        n_tokens=1,
                    )
                nc.vector.tensor_relu(hcur[:], lno[:])
                if not last:
                    # committed h also goes into history^T (relu fused in ACT)
                    nc.scalar.activation(
                        HT[:, :, :, t],
                        lno.rearrange("p b f -> p f b"),
                        mybir.ActivationFunctionType.Relu,
                    )

            def append_rows(t):
                # Hs[(b, t), (p, f')] = hcur[p, b, f']  (one small DMA per b)
                for b in range(BL):
                    r = b * S + t
                    nc.sync.dma_start(Hs[r : r + 1], hcur[:, b, :])

            for t in range(T):
                last = t == T - 1
                if t == 0:
                    x0 = xpool.tile([P, BL, F], FP32, tag="x")
                    nc.vector.tensor_copy(x0[:], zw[:, 0, :, :])
                    layer_norm_relu(x0, t, last)
                    append_rows(t)
                    continue

                # h_base^T = W_h h_{t-1} (+ zw added on psum->sbuf copy)
                px = pxp.tile([P, F, BL], FP32, tag="px")
                for fp in range(F):
                    for fk in range(F):
                        # start arms zero-on-first-touch for the whole 2KB
                        # psum region: exactly one start per px lifetime.
                        nc.tensor.matmul(
                            px[:, fp, :],
                            wh[:, fk, fp, :],
                            hcur[:, :, fk],
                            start=(fp == 0 and fk == 0),
                            stop=(fp == F - 1 and fk == F - 1),
                        )
                x = xpool.tile([P, BL, F], FP32, tag="x")
                nc.vector.tensor_add(x[:], px.rearrange("p f b -> p b f"), zw[:, t, :, :])
                layer_norm_relu(x, t, True)  # k=0: do not commit yet

                c_prev = None
                for k in range(S_LOOP):
                    lastk = k == S_LOOP - 1
                    # c^T[(b,s), b'] = sum_j H^T[j,(b,s)] h[j, b']
                    pct = pcp.tile([P, BL], FP32, tag="pct")
                    for f in range(F):
                        nc.tensor.matmul(
                            pct[:],
                            HT[:, f, :, :],
                            hcur[:, :, f],
                            start=(f == 0),
                            stop=(f == F - 1),
                        )
                    ck = cpool.tile([P, BL], FP32, tag="ck")
                    nc.vector.tensor_mul(ck[:], pct[:], msk[:, t, :])
                    if c_prev is None:
                        rhs = ck
                    else:
                        dp = cpool.tile([P, BL], FP32, tag="dp")
                        nc.vector.tensor_sub(dp[:], ck[:], c_prev[:])
                        rhs = dp
                    c_prev = ck
                    # x^T += Ah^T (delta form so px keeps h_base + latest Ah)
                    for fp in range(F):
                        nc.tensor.matmul(
                            px[:, fp, :],
                            Hs[:, :, fp],
                            rhs[:],
                            start=False,
                            stop=(fp == F - 1),
                            skip_group_check=True,
                        )
                    xk = xpool.tile([P, BL, F], FP32, tag="x")
                    nc.vector.tensor_add(xk[:], px.rearrange("p f b -> p b f"), zw[:, t, :, :])
                    layer_norm_relu(xk, t, last or not lastk)

                if not last:
                    append_rows(t)

            nc.sync.dma_start(out_d[:], hcur[:])

    nc.compile()
    return nc


def _host_prep(z_seq, W_h, W_g, b_h, ln_gamma, ln_beta):
    """Build the per-core input maps (all layout shuffling happens here)."""
    z_seq = np.asarray(z_seq, np.float32)
    W_h = np.ascontiguousarray(np.asarray(W_h, np.float32))
    W_g = np.ascontiguousarray(np.asarray(W_g, np.float32))
    b_h = np.asarray(b_h, np.float32)
    ln_gamma = np.asarray(ln_gamma, np.float32)
    ln_beta = np.asarray(ln_beta, np.float32)

    # lhsT tiles: wh[pk, f_k, f', p] = W_h[p*4+f', pk*4+f_k]
    wh = np.ascontiguousarray(
        W_h.reshape(P, F, P, F).transpose(2, 3, 1, 0)
    )
    # wg[gg, gc, f', p] = W_g[p*4+f', gc*128+gg]
    wg = np.ascontiguousarray(
        W_g.reshape(P, F, 2, P).transpose(3, 2, 1, 0)
    )
    bh = np.ascontiguousarray(b_h.reshape(P, F))
    gam = np.ascontiguousarray(ln_gamma.reshape(P, F))
    bet = np.ascontiguousarray(ln_beta.reshape(P, F))

    # maskw[(b,s), (t, b')] = (b==b') * eta * lambda^(t-1-s) for s<t else 0
    msk = np.zeros((BL, S, T, BL), np.float64)
    for t in range(1, T):
        s = np.arange(t)
        w = ETA * LAMBDA ** (t - 1 - s)
        for b in range(BL):
            msk[b, :t, t, b] = w
    msk = np.ascontiguousarray(msk.reshape(P, T, BL).astype(np.float32))

    in_maps = []
    for c in range(N_CORES):
        zl = z_seq[:, c * BL : (c + 1) * BL, :]  # [T, BL, DG]
        # zt[gg, gc, t, b] = z[t, b, gc*128+gg]
        zt = np.ascontiguousarray(
            zl.transpose(2, 0, 1).reshape(2, P, T, BL).transpose(1, 0, 2, 3)
        )
        in_maps.append(
            {
                "wh": wh, "wg": wg, "zt": zt, "bh": bh,
                "gam": gam, "bet": bet, "msk": msk,
            }
        )
    return in_maps


def _make_runner():
    """Cached jitted runner (mirrors bass2jax.run_bass_via_pjrt multi-core
    path, but keeps the jitted executable across calls)."""
    import jax
    from jax.sharding import Mesh, PartitionSpec
    from jax.experimental.shard_map import shard_map
    from concourse import bass2jax as b2j
    import concourse.mybir as mb

    nc = _build()
    b2j.install_neuronx_cc_hook()

    partition_name = nc.partition_id_tensor.name if nc.partition_id_tensor else None
    in_names, out_names, out_avals, zero_outs = [], [], [], []
    for alloc in nc.m.functions[0].allocations:
        if not isinstance(mb.MemoryLocationSet, type) or not isinstance(alloc, mb.MemoryLocationSet):
            continue
        name = alloc.memorylocations[0].name
        if alloc.kind == "ExternalInput":
            if name != partition_name:
                in_names.append(name)
        elif alloc.kind == "ExternalOutput":
            shape = tuple(alloc.tensor_shape)
            dtype = mb.dt.np(alloc.dtype)
            out_names.append(name)
            out_avals.append(jax.core.ShapedArray(shape, dtype))
            zero_outs.append(np.zeros(shape, dtype))
    n_params = len(in_names)
    n_outs = len(out_avals)
    all_in_names = list(in_names) + list(out_names)
    if partition_name is not None:
        all_in_names.append(partition_name)

    donate = tuple(range(n_params, n_params + n_outs))

    def _body(*args):
        operands = list(args)
        if partition_name is not None:
            operands.append(b2j.partition_id_tensor())
        outs = b2j._bass_exec_p.bind(
            *operands,
            out_avals=tuple(out_avals),
            in_names=tuple(all_in_names),
            out_names=tuple(out_names),
            lowering_input_output_aliases=(),
            sim_require_finite=True,
            sim_require_nnan=True,
            nc=nc,
        )
        return tuple(outs)

    devices = jax.devices()[:N_CORES]
    mesh = Mesh(np.asarray(devices), ("core",))
    in_specs = (PartitionSpec("core"),) * (n_params + n_outs)
    out_specs = (PartitionSpec("core"),) * n_outs
    sharded = jax.jit(
        shard_map(_body, mesh=mesh, in_specs=in_specs, out_specs=out_specs,
                  check_rep=False),
        donate_argnums=donate,
        keep_unused=True,
    )

    def run(in_maps):
        concat_in = [
            np.concatenate([np.asarray(in_maps[c][nm]) for c in range(N_CORES)], axis=0)
            for nm in in_names
        ]
        concat_zeros = [
            np.zeros((N_CORES * z.shape[0], *z.shape[1:]), z.dtype) for z in zero_outs
        ]
        out_arrs = sharded(*concat_in, *concat_zeros)
        return [
            {
                nm: np.asarray(out_arrs[i]).reshape(N_CORES, *out_avals[i].shape)[c]
                for i, nm in enumerate(out_names)
            }
            for c in range(N_CORES)
        ]

    return run


def kernel(z_seq, W_h, W_g, b_h, ln_gamma, ln_beta):
    global _cached
    if _cached is None:
        _cached = _make_runner()
    run = _cached
    in_maps = _host_prep(z_seq, W_h, W_g, b_h, ln_gamma, ln_beta)
    results = run(in_maps)
    outs = []
    for c in range(N_CORES):
        raw = results[c]["out"]  # [P, BL, F]: raw[p, b, f] = h[b, p*4+f]
        outs.append(raw.transpose(1, 0, 2).reshape(BL, DH))
    return np.ascontiguousarray(np.concatenate(outs, axis=0).astype(np.float32))
